# revision 12
# baseline (speedup 1.0000x reference)
"""Trainium2 Bass kernel for NNBlendFM: 3-layer tanh MLP embedder + 64-head
rank-16 factorization machine, data-parallel over batch across 8 NeuronCores.

Math (per batch row b, head h):
    h = tanh(tanh(tanh(x W1 + b1) W2 + b2) W3 + b3)          # [B, 2048]
    lin[b,h]  = h . fm_w[h]
    vx[b,h,r] = h . fm_V[h,r]
    diag[b,h] = (h*h) . (sum_r fm_V[h,r]^2)
    out[h,b]  = fm_w0[h] + lin + 0.5*(sum_r vx^2 - diag)

Device layout: activations kept as [feature_partition, batch_free] tiles so
every matmul contracts over the partition dim with natural-layout weights as
the stationary operand.  The FM stage flips to [batch_partition, col_free] by
using h^T k-tiles as the stationary operand.  All matmul inputs are bf16
(fp32 PSUM accumulation), everything else fp32.

Head schedule: the kernel is HBM-latency-bound for its first ~16us (x+W1 are
3MB and DMA aggregate is ~410 GB/s, with ~8.4us of fixed prologue+queue
spin-up before the first packet lands).  To hide it, x and W1 are split into
24 x 128KB row-chunks issued k-ascending round-robin over four engine DMA
queues, and layer 1 runs k-outer over jt-groups of 4 (8 open PSUM banks) so
the PE consumes each k-tile as it lands instead of waiting for all of them.
"""

import numpy as np
import ml_dtypes

import concourse.tile as tile
from concourse import bacc, mybir
from concourse import bass_utils

BF16 = mybir.dt.bfloat16
F32 = mybir.dt.float32
AF = mybir.ActivationFunctionType
ALU = mybir.AluOpType

P = 128
IN, HID, HEADS, RANK = 512, 2048, 64, 16
B = 8192
NCORES = 8
BC = B // NCORES            # 1024 batch rows per core
KT1 = IN // P               # 4  k-tiles, layer 1
KT = HID // P               # 16 k-tiles, layers 2/3 + FM
JT = HID // P               # 16 output-feature tiles per layer
NB = 512                    # matmul moving free-dim (one PSUM bank)
NBC = BC // NB              # 2 batch column chunks
BT = BC // P                # 8 batch tiles in FM stage
HR = HEADS * RANK           # 1024 vx columns
WARMUP_MM = 8               # PE warm-up matmuls (HAM ramp coverage)

_CACHE = {}


def _build_module():
    nc = bacc.Bacc(
        "TRN2", target_bir_lowering=False, debug=False, num_devices=NCORES
    )
    dt = nc.dram_tensor
    xT = dt("xT", [IN, BC], BF16, kind="ExternalInput").ap()
    # W1 in column-group-major DRAM layout: row g*IN + r holds
    # W1[r, g*512 : (g+1)*512] so the (k, g) chunk the head streams is a
    # contiguous 128KB block (4KB coalesced packets).
    W1 = dt("W1", [4 * IN, HID // 4], BF16, kind="ExternalInput").ap()
    W2 = dt("W2", [HID, HID], BF16, kind="ExternalInput").ap()
    W3 = dt("W3", [HID, HID], BF16, kind="ExternalInput").ap()
    B1 = dt("B1", [P, JT], F32, kind="ExternalInput").ap()
    B2 = dt("B2", [P, JT], F32, kind="ExternalInput").ap()
    B3 = dt("B3", [P, JT], F32, kind="ExternalInput").ap()
    VT = dt("VT", [HID, HR], BF16, kind="ExternalInput").ap()
    FW = dt("FW", [P, KT * HEADS], BF16, kind="ExternalInput").ap()
    SQ = dt("SQ", [P, KT * HEADS], BF16, kind="ExternalInput").ap()
    W0C = dt("W0C", [P, HEADS], BF16, kind="ExternalInput").ap()
    OUT = dt("out", [BC, HEADS], F32, kind="ExternalOutput").ap()

    with tile.TileContext(nc) as tc:
        with (
            tc.tile_pool(name="wpool", bufs=24) as wpool,
            tc.tile_pool(name="hpool", bufs=32) as hpool,
            tc.tile_pool(name="vtpool", bufs=16) as vtpool,
            tc.tile_pool(name="cpool", bufs=1) as cpool,
            tc.tile_pool(name="pp", bufs=8, space="PSUM") as pp,
            tc.tile_pool(name="epool", bufs=2) as epool,
            tc.tile_pool(name="spool", bufs=8) as spool,
            tc.tile_pool(name="opool", bufs=4) as opool,
        ):
            # PE warm-up: dummy matmuls on a zeroed borrowed tile keep the PE
            # busy through the DMA head so HAM un-throttles (1.2 -> 2.4 GHz)
            # before the first real matmul.  vt0 is borrowed — its real DMA
            # fill happens mid-kernel, long after the warm-up reads.
            vtt = []
            for k in range(KT):
                vt_k = vtpool.tile([P, HR], BF16, tag="vt", name=f"vt{k}")
                vtt.append(vt_k)
            wsrc = vtt[0][:, 0:NB]
            nc.gpsimd.memset(wsrc, 0.0)
            wu = pp.tile([P, NB], F32, tag="ps", name="warm")
            for _ in range(WARMUP_MM):
                nc.tensor.matmul(
                    wu[:], wsrc[:, 0:P], wsrc[:], start=True, stop=True
                )

            # --- critical-path head DMA: x + W1 as 24 x 128KB chunks -------
            # Only sync/scalar/gpsimd can issue DMAs; sync+scalar are the
            # fast HWDGE queues, gpsimd's software DGE starts ~2us later and
            # sustains less.  Layer 1's group-0 critical set is only x (1MB)
            # + W1's g=0 column blocks (512KB) thanks to the g-major W1
            # layout; its 6 chunks ride sync+scalar in consumption order so
            # k-tile i of group 0 is resident ~1.7us after k-tile i-1.
            # b1 goes FIRST on gpsimd (8KB) so the first tanh never waits.
            b1t = cpool.tile([P, JT], F32, tag="b1")
            nc.gpsimd.dma_start(b1t[:], B1)
            onest = cpool.tile([P, P], BF16, tag="ones")
            nc.gpsimd.memset(onest[:], 1.0)
            xt = []
            w1t = []
            for k in range(KT1):
                x_k = hpool.tile([P, BC], BF16, tag="h", name=f"xt{k}")
                xt.append(x_k)
                w_k = wpool.tile([P, HID], BF16, tag="w", name=f"w1_{k}")
                w1t.append(w_k)

            def w1_chunk(eng, k, g):
                eng.dma_start(
                    w1t[k][:, g * 512 : (g + 1) * 512],
                    W1[g * IN + k * P : g * IN + (k + 1) * P, :],
                )

            # g=0 critical chunks, alternating queues per k.
            nc.sync.dma_start(xt[0][:], xT[0:P, :])
            w1_chunk(nc.scalar, 0, 0)
            w1_chunk(nc.sync, 1, 0)
            nc.scalar.dma_start(xt[1][:], xT[P : 2 * P, :])
            nc.sync.dma_start(xt[2][:], xT[2 * P : 3 * P, :])
            w1_chunk(nc.scalar, 2, 0)
            w1_chunk(nc.sync, 3, 0)
            nc.scalar.dma_start(xt[3][:], xT[3 * P : 4 * P, :])
            # W1 g=1 split sync/scalar (lands ~15.5us, consumed ~18us);
            # g=2/3 ride gpsimd's early window (lands ~14-18us).
            w1_chunk(nc.scalar, 0, 1)
            w1_chunk(nc.scalar, 1, 1)
            w1_chunk(nc.sync, 2, 1)
            w1_chunk(nc.sync, 3, 1)
            # b2 on scalar (tiny); scalar then stays ACT-only for tanh.
            b2t = cpool.tile([P, JT], F32, tag="b2")
            nc.scalar.dma_start(b2t[:], B2)
            for gg in range(2, 4):
                for k in range(KT1):
                    w1_chunk(nc.gpsimd, k, gg)

            # W2: k15..k13 on gpsimd (trickles until ~33us), k0..k12 on sync
            # right behind its 1.5MB head (~17-40us at full BW).  L2 consumes
            # k in arrival order (see L2_BASE below).
            w2t = [None] * KT
            for k in range(15, 12, -1):
                w_k = wpool.tile([P, HID], BF16, tag="w", name=f"w2_{k}")
                nc.gpsimd.dma_start(w_k[:], W2[k * P : (k + 1) * P, :])
                w2t[k] = w_k
            for k in range(13):
                w_k = wpool.tile([P, HID], BF16, tag="w", name=f"w2_{k}")
                nc.sync.dma_start(w_k[:], W2[k * P : (k + 1) * P, :])
                w2t[k] = w_k
            # W3 all on sync (needed from ~148us, lands by ~60us).
            w3t = []
            for k in range(KT):
                w_k = wpool.tile([P, HID], BF16, tag="w", name=f"w3_{k}")
                nc.sync.dma_start(w_k[:], W3[k * P : (k + 1) * P, :])
                w3t.append(w_k)

            # FM operands on gpsimd behind its W2 share (needed ~257us).
            for k in range(KT):
                nc.gpsimd.dma_start(vtt[k][:], VT[k * P : (k + 1) * P, :])
            b3t = cpool.tile([P, JT], F32, tag="b3")
            nc.gpsimd.dma_start(b3t[:], B3)
            # -w0/128 replicated; contracted against a ones column block so
            # the diag PSUM group finishes as (0.5*diag - w0).
            w0c = cpool.tile([P, HEADS], BF16, tag="w0c")
            nc.gpsimd.dma_start(w0c[:], W0C)
            fwt = cpool.tile([P, KT * HEADS], BF16, tag="fw")
            nc.gpsimd.dma_start(fwt[:], FW)
            sqt = cpool.tile([P, KT * HEADS], BF16, tag="sq")
            nc.gpsimd.dma_start(sqt[:], SQ)

            # --- layer 1: k-outer over jt-groups of 4 (8 open PSUM banks) --
            # The PE starts on k-tile 0 as soon as it lands and absorbs each
            # later k-tile as it arrives; a jt-major loop would stall until
            # ALL of x+W1 were resident.
            h1 = [None] * JT
            for grp in range(JT // 4):
                jts = list(range(4 * grp, 4 * grp + 4))
                ps = {}
                for jt in jts:
                    ps[jt] = [
                        pp.tile([P, NB], F32, tag="ps", name=f"l1ps{jt}_{c}")
                        for c in range(NBC)
                    ]
                for k in range(KT1):
                    for jt in jts:
                        lhsT = w1t[k][:, jt * P : (jt + 1) * P]
                        for c in range(NBC):
                            nc.tensor.matmul(
                                ps[jt][c][:],
                                lhsT,
                                xt[k][:, c * NB : (c + 1) * NB],
                                start=(k == 0),
                                stop=(k == KT1 - 1),
                            )
                for jt in jts:
                    ht = hpool.tile([P, BC], BF16, tag="h", name=f"l1h{jt}")
                    for c in range(NBC):
                        nc.scalar.activation(
                            ht[:, c * NB : (c + 1) * NB],
                            ps[jt][c][:],
                            AF.Tanh,
                            bias=b1t[:, jt : jt + 1],
                        )
                    h1[jt] = ht

            # --- layers 2/3: jt-outer with rotated k-accumulation ----------
            # L2's base k-order follows W2 DMA arrival (gpsimd's k15..k13
            # land first, then sync's k0..k12) so jt=0 never waits.
            L2_BASE = [15, 14, 13] + list(range(13))

            def layer(h_prev, w_tiles, bias_t, ktiles, name, base=None):
                if base is None:
                    base = list(range(ktiles))
                h_out = []
                for jt in range(JT):
                    ps = []
                    for c in range(NBC):
                        ps_c = pp.tile([P, NB], F32, tag="ps", name=f"{name}ps{jt}_{c}")
                        ps.append(ps_c)
                    # Rotate the accumulation order by jt so each weight
                    # tile's final read retires early for some jt, releasing
                    # its pool slot for the next layer's prefetch DMA.
                    kts = [base[(i + jt) % ktiles] for i in range(ktiles)]
                    for i, kt in enumerate(kts):
                        lhsT = w_tiles[kt][:, jt * P : (jt + 1) * P]
                        for c in range(NBC):
                            nc.tensor.matmul(
                                ps[c][:],
                                lhsT,
                                h_prev[kt][:, c * NB : (c + 1) * NB],
                                start=(i == 0),
                                stop=(i == ktiles - 1),
                            )
                    ht = hpool.tile([P, BC], BF16, tag="h", name=f"{name}h{jt}")
                    for c in range(NBC):
                        nc.scalar.activation(
                            ht[:, c * NB : (c + 1) * NB],
                            ps[c][:],
                            AF.Tanh,
                            bias=bias_t[:, jt : jt + 1],
                        )
                    h_out.append(ht)
                return h_out

            h2 = layer(h1, w2t, b2t, KT, "l2", base=L2_BASE)
            h3 = layer(h2, w3t, b3t, KT, "l3")

            # --- h3 squared (stationary operand for the diag matmuls) -----
            h3sq = []
            for k in range(KT):
                sq_k = hpool.tile([P, BC], BF16, tag="h", name=f"h3sq{k}")
                nc.vector.tensor_mul(sq_k[:], h3[k][:], h3[k][:])
                h3sq.append(sq_k)

            # --- FM stage: per 128-row batch tile -------------------------
            def fm_phase_a(bt):
                """vx = h V^T (1024 cols) and lin = h fm_w^T (64 cols)."""
                vx0 = pp.tile([P, NB], F32, tag="ps", name=f"vx0_{bt}")
                vx1 = pp.tile([P, NB], F32, tag="ps", name=f"vx1_{bt}")
                lw = pp.tile([P, NB], F32, tag="ps", name=f"lw_{bt}")
                bsl = slice(bt * P, (bt + 1) * P)
                for kt in range(KT):
                    lhsT = h3[kt][:, bsl]
                    nc.tensor.matmul(
                        vx0[:], lhsT, vtt[kt][:, 0:NB],
                        start=(kt == 0), stop=(kt == KT - 1),
                    )
                    nc.tensor.matmul(
                        vx1[:], lhsT, vtt[kt][:, NB:HR],
                        start=(kt == 0), stop=(kt == KT - 1),
                    )
                    nc.tensor.matmul(
                        lw[:, 0:HEADS], lhsT,
                        fwt[:, kt * HEADS : (kt + 1) * HEADS],
                        start=(kt == 0), stop=(kt == KT - 1),
                    )
                return vx0, vx1, lw

            def fm_phase_b(bt):
                """diag = (h*h) . (0.5 * sum_r V^2), already scaled by 0.5."""
                dg = pp.tile([P, NB], F32, tag="ps", name=f"dg_{bt}")
                bsl = slice(bt * P, (bt + 1) * P)
                for kt in range(KT):
                    nc.tensor.matmul(
                        dg[:, 0:HEADS],
                        h3sq[kt][:, bsl],
                        sqt[:, kt * HEADS : (kt + 1) * HEADS],
                        start=(kt == 0), stop=False,
                    )
                nc.tensor.matmul(
                    dg[:, 0:HEADS], onest[:], w0c[:], start=False, stop=True,
                )
                return dg

            def fm_square_reduce(bt, vx0, vx1):
                """Emitted right after phase A: overlaps later bt's matmuls.
                Each 512-wide half squares then reduces independently so the
                two chains pipeline across ACT and DVE."""
                vx2 = epool.tile([P, HR], F32, tag="e", name=f"vx2_{bt}")
                sumv = spool.tile([P, HEADS], F32, tag="s", name=f"sumv_{bt}")
                for c, vxh in ((0, vx0), (1, vx1)):
                    nc.scalar.activation(vx2[:, c * NB : (c + 1) * NB], vxh[:], AF.Square)
                    nc.vector.reduce_sum(
                        sumv[:, c * (HEADS // 2) : (c + 1) * (HEADS // 2)],
                        vx2[:, c * NB : (c + 1) * NB].rearrange(
                            "p (h r) -> p h r", r=RANK
                        ),
                        axis=mybir.AxisListType.X,
                    )
                return sumv

            def fm_combine(bt, sumv, lw, dg):
                # q = 0.5*sumv - diag_half
                q = spool.tile([P, HEADS], F32, tag="s", name=f"q_{bt}")
                nc.vector.scalar_tensor_tensor(
                    q[:], sumv[:], 0.5, dg[:, 0:HEADS],
                    op0=ALU.mult, op1=ALU.subtract,
                )
                ot = opool.tile([P, HEADS], F32, tag="o", name=f"ot_{bt}")
                nc.vector.tensor_add(ot[:], q[:], lw[:, 0:HEADS])
                nc.sync.dma_start(OUT[bt * P : (bt + 1) * P, :], ot[:])

            # Stagger: A(0), A(1), B(0), C(0), A(2), B(1), C(1), ...
            pend = []  # (bt, sumv, lw)
            for bt in range(BT):
                vx0, vx1, lw = fm_phase_a(bt)
                sumv = fm_square_reduce(bt, vx0, vx1)
                pend.append((bt, sumv, lw))
                if len(pend) == 2:
                    obt, osumv, olw = pend.pop(0)
                    dg = fm_phase_b(obt)
                    fm_combine(obt, osumv, olw, dg)
            while pend:
                obt, osumv, olw = pend.pop(0)
                dg = fm_phase_b(obt)
                fm_combine(obt, osumv, olw, dg)

    nc.compile()
    return nc


def _get_nc():
    if "nc" not in _CACHE:
        _CACHE["nc"] = _build_module()
    return _CACHE["nc"]


def _prep_host(x, W1, b1, W2, b2, W3, b3, fm_w0, fm_w, fm_V):
    """Host-side layout prep: bf16 casts, transposes, per-head V reductions."""
    bf = ml_dtypes.bfloat16
    f32 = np.float32

    common = {
        # column-group-major: row g*512 + r = W1[r, g*512:(g+1)*512]
        "W1": np.ascontiguousarray(
            W1.reshape(IN, 4, HID // 4).transpose(1, 0, 2).reshape(4 * IN, HID // 4)
            .astype(bf)
        ),
        "W2": np.ascontiguousarray(W2.astype(bf)),
        "W3": np.ascontiguousarray(W3.astype(bf)),
        "B1": np.ascontiguousarray(b1.astype(f32).reshape(JT, P).T),
        "B2": np.ascontiguousarray(b2.astype(f32).reshape(JT, P).T),
        "B3": np.ascontiguousarray(b3.astype(f32).reshape(JT, P).T),
        # V^T: [2048, heads*rank], col hr = h*RANK + r
        "VT": np.ascontiguousarray(
            fm_V.reshape(HEADS * RANK, HID).T.astype(bf)
        ),
        # fm_w^T packed as [128, kt*64]: FW[p, kt*64+h] = fm_w[h, kt*128+p]
        "FW": np.ascontiguousarray(
            fm_w.T.reshape(KT, P, HEADS).transpose(1, 0, 2).reshape(P, KT * HEADS)
            .astype(bf)
        ),
        # 0.5 * sum_r V^2, same packing
        "SQ": np.ascontiguousarray(
            (0.5 * (fm_V.astype(np.float64) ** 2).sum(axis=1))
            .T.reshape(KT, P, HEADS).transpose(1, 0, 2).reshape(P, KT * HEADS)
            .astype(bf)
        ),
        "W0C": np.ascontiguousarray(
            np.tile((-fm_w0.astype(np.float64) / P)[None, :], (P, 1))
            .astype(ml_dtypes.bfloat16)
        ),
    }

    in_maps = []
    xb = x.astype(bf)
    for c in range(NCORES):
        m = dict(common)
        m["xT"] = np.ascontiguousarray(xb[c * BC : (c + 1) * BC, :].T)
        in_maps.append(m)
    return in_maps


def kernel(x, W1, b1, W2, b2, W3, b3, fm_w0, fm_w, fm_V):
    # Host prep is plain numpy; coerce eagerly in case inputs are jax arrays.
    x, W1, b1, W2, b2, W3, b3, fm_w0, fm_w, fm_V = (
        np.asarray(a) for a in (x, W1, b1, W2, b2, W3, b3, fm_w0, fm_w, fm_V)
    )
    nc = _get_nc()
    in_maps = _prep_host(x, W1, b1, W2, b2, W3, b3, fm_w0, fm_w, fm_V)
    import os
    trace = bool(int(os.environ.get("KERNEL_TRACE", "0")))
    last_err = None
    for _attempt in range(3):
        try:
            res = bass_utils.run_bass_kernel_spmd(
                nc, in_maps, core_ids=list(range(NCORES)), trace=trace,
            )
            outs = [np.asarray(res.results[c]["out"]) for c in range(NCORES)]
            break
        except Exception as e:  # transient device faults (NRT unrecoverable)
            last_err = e
    else:
        raise last_err
    _CACHE["last_results"] = res
    full = np.concatenate(outs, axis=0)          # [B, HEADS]
    return np.ascontiguousarray(full.T).astype(np.float32)  # [HEADS, B]


# revision 16
# speedup vs baseline: 1.1978x; 1.1978x over previous
"""Trainium2 Bass kernel for NNBlendFM: 3-layer tanh MLP embedder + 64-head
rank-16 factorization machine, data-parallel over batch across 8 NeuronCores.

Math (per batch row b, head h):
    h = tanh(tanh(tanh(x W1 + b1) W2 + b2) W3 + b3)          # [B, 2048]
    lin[b,h]  = h . fm_w[h]
    vx[b,h,r] = h . fm_V[h,r]
    diag[b,h] = (h*h) . (sum_r fm_V[h,r]^2)
    out[h,b]  = fm_w0[h] + lin + 0.5*(sum_r vx^2 - diag)

Device layout: activations kept as [feature_partition, batch_free] tiles so
every matmul contracts over the partition dim with natural-layout weights as
the stationary operand.  The FM stage flips to [batch_partition, col_free] by
using h^T k-tiles as the stationary operand.  All matmul inputs are bf16
(fp32 PSUM accumulation), everything else fp32.

Head schedule: the kernel is HBM-latency-bound for its first ~16us (x+W1 are
3MB and DMA aggregate is ~410 GB/s, with ~8.4us of fixed prologue+queue
spin-up before the first packet lands).  To hide it, x and W1 are split into
24 x 128KB row-chunks issued k-ascending round-robin over four engine DMA
queues, and layer 1 runs k-outer over jt-groups of 4 (8 open PSUM banks) so
the PE consumes each k-tile as it lands instead of waiting for all of them.
"""

import numpy as np
import ml_dtypes

import concourse.tile as tile
from concourse import bacc, mybir
from concourse import bass_utils

BF16 = mybir.dt.bfloat16
F32 = mybir.dt.float32
AF = mybir.ActivationFunctionType
ALU = mybir.AluOpType

P = 128
IN, HID, HEADS, RANK = 512, 2048, 64, 16
B = 8192
NCORES = 8
BC = B // NCORES            # 1024 batch rows per core
KT1 = IN // P               # 4  k-tiles, layer 1
KT = HID // P               # 16 k-tiles, layers 2/3 + FM
JT = HID // P               # 16 output-feature tiles per layer
NB = 512                    # matmul moving free-dim (one PSUM bank)
NBC = BC // NB              # 2 batch column chunks
BT = BC // P                # 8 batch tiles in FM stage
HR = HEADS * RANK           # 1024 vx columns
WARMUP_MM = 8               # PE warm-up matmuls (HAM ramp coverage)

_CACHE = {}


def _build_module():
    nc = bacc.Bacc(
        "TRN2", target_bir_lowering=False, debug=False, num_devices=NCORES
    )
    dt = nc.dram_tensor
    xT = dt("xT", [IN, BC], BF16, kind="ExternalInput").ap()
    # W1 in column-group-major DRAM layout: row g*IN + r holds
    # W1[r, g*512 : (g+1)*512] so the (k, g) chunk the head streams is a
    # contiguous 128KB block (4KB coalesced packets).
    W1 = dt("W1", [4 * IN, HID // 4], BF16, kind="ExternalInput").ap()
    W2 = dt("W2", [HID, HID], BF16, kind="ExternalInput").ap()
    W3 = dt("W3", [HID, HID], BF16, kind="ExternalInput").ap()
    B1 = dt("B1", [P, JT], F32, kind="ExternalInput").ap()
    B2 = dt("B2", [P, JT], F32, kind="ExternalInput").ap()
    B3 = dt("B3", [P, JT], F32, kind="ExternalInput").ap()
    VT = dt("VT", [HID, HR], BF16, kind="ExternalInput").ap()
    FW = dt("FW", [P, KT * HEADS], BF16, kind="ExternalInput").ap()
    SQ = dt("SQ", [P, KT * HEADS], BF16, kind="ExternalInput").ap()
    W0C = dt("W0C", [P, HEADS], BF16, kind="ExternalInput").ap()
    OUT = dt("out", [BC, HEADS], F32, kind="ExternalOutput").ap()

    with tile.TileContext(nc) as tc:
        with (
            tc.tile_pool(name="wpool", bufs=24) as wpool,
            tc.tile_pool(name="hpool", bufs=32) as hpool,
            tc.tile_pool(name="vtpool", bufs=16) as vtpool,
            tc.tile_pool(name="cpool", bufs=1) as cpool,
            tc.tile_pool(name="pp", bufs=8, space="PSUM") as pp,
            tc.tile_pool(name="epool", bufs=2) as epool,
            tc.tile_pool(name="spool", bufs=8) as spool,
            tc.tile_pool(name="opool", bufs=4) as opool,
        ):
            # PE warm-up: dummy matmuls on a zeroed borrowed tile keep the PE
            # busy through the DMA head so HAM un-throttles (1.2 -> 2.4 GHz)
            # before the first real matmul.  vt0 is borrowed — its real DMA
            # fill happens mid-kernel, long after the warm-up reads.
            vtt = []
            for k in range(KT):
                vt_k = vtpool.tile([P, HR], BF16, tag="vt", name=f"vt{k}")
                vtt.append(vt_k)
            wsrc = vtt[0][:, 0:NB]
            nc.gpsimd.memset(wsrc, 0.0)
            wu = pp.tile([P, NB], F32, tag="ps", name="warm")
            for _ in range(WARMUP_MM):
                nc.tensor.matmul(
                    wu[:], wsrc[:, 0:P], wsrc[:], start=True, stop=True
                )

            # --- critical-path head DMA: x + W1 as 24 x 128KB chunks -------
            # Only sync/scalar/gpsimd can issue DMAs; sync+scalar are the
            # fast HWDGE queues, gpsimd's software DGE starts ~2us later and
            # sustains less.  Layer 1's group-0 critical set is only x (1MB)
            # + W1's g=0 column blocks (512KB) thanks to the g-major W1
            # layout; its 6 chunks ride sync+scalar in consumption order so
            # k-tile i of group 0 is resident ~1.7us after k-tile i-1.
            # b1 goes FIRST on gpsimd (8KB) so the first tanh never waits.
            b1t = cpool.tile([P, JT], F32, tag="b1")
            nc.gpsimd.dma_start(b1t[:], B1)
            onest = cpool.tile([P, P], BF16, tag="ones")
            nc.gpsimd.memset(onest[:], 1.0)
            xt = []
            w1t = []
            for k in range(KT1):
                x_k = hpool.tile([P, BC], BF16, tag="h", name=f"xt{k}")
                xt.append(x_k)
                w_k = wpool.tile([P, HID], BF16, tag="w", name=f"w1_{k}")
                w1t.append(w_k)

            def w1_chunk(eng, k, g):
                eng.dma_start(
                    w1t[k][:, g * 512 : (g + 1) * 512],
                    W1[g * IN + k * P : g * IN + (k + 1) * P, :],
                )

            # g=0 critical chunks, alternating queues per k.
            nc.sync.dma_start(xt[0][:], xT[0:P, :])
            w1_chunk(nc.scalar, 0, 0)
            w1_chunk(nc.sync, 1, 0)
            nc.scalar.dma_start(xt[1][:], xT[P : 2 * P, :])
            nc.sync.dma_start(xt[2][:], xT[2 * P : 3 * P, :])
            w1_chunk(nc.scalar, 2, 0)
            w1_chunk(nc.sync, 3, 0)
            nc.scalar.dma_start(xt[3][:], xT[3 * P : 4 * P, :])
            # W1 g=1 split sync/scalar (lands ~15.5us, consumed ~18us);
            # g=2/3 ride gpsimd's early window (lands ~14-18us).
            w1_chunk(nc.scalar, 0, 1)
            w1_chunk(nc.scalar, 1, 1)
            w1_chunk(nc.sync, 2, 1)
            w1_chunk(nc.sync, 3, 1)
            # b2 on scalar (tiny); scalar then stays ACT-only for tanh.
            b2t = cpool.tile([P, JT], F32, tag="b2")
            nc.scalar.dma_start(b2t[:], B2)
            for gg in range(2, 4):
                for k in range(KT1):
                    w1_chunk(nc.gpsimd, k, gg)

            # W2: k15..k13 on gpsimd (trickles until ~30us), k0..k10 on sync
            # right behind its 1.5MB head (~17-40us); k11/k12 issue from
            # scalar mid-L1 (see below).  L2 consumes k in arrival order
            # (see L2_BASE below).
            w2t = [None] * KT
            for k in range(15, 12, -1):
                w_k = wpool.tile([P, HID], BF16, tag="w", name=f"w2_{k}")
                nc.gpsimd.dma_start(w_k[:], W2[k * P : (k + 1) * P, :])
                w2t[k] = w_k
            for k in range(13):
                w_k = wpool.tile([P, HID], BF16, tag="w", name=f"w2_{k}")
                nc.sync.dma_start(w_k[:], W2[k * P : (k + 1) * P, :])
                w2t[k] = w_k
            # Small FM constants finish gpsimd's share (needed ~257us).
            b3t = cpool.tile([P, JT], F32, tag="b3")
            nc.gpsimd.dma_start(b3t[:], B3)
            # -w0/128 replicated; contracted against a ones column block so
            # the diag PSUM group finishes as (0.5*diag - w0).
            w0c = cpool.tile([P, HEADS], BF16, tag="w0c")
            nc.gpsimd.dma_start(w0c[:], W0C)
            fwt = cpool.tile([P, KT * HEADS], BF16, tag="fw")
            nc.gpsimd.dma_start(fwt[:], FW)
            sqt = cpool.tile([P, KT * HEADS], BF16, tag="sq")
            nc.gpsimd.dma_start(sqt[:], SQ)
            # W3 then VT on sync: W3 lands ~40-60us (needed from ~148us),
            # VT ~60-80us (needed ~257us) — sync is the lone bulk queue by
            # then and sustains ~400 GB/s.
            w3t = []
            for k in range(KT):
                w_k = wpool.tile([P, HID], BF16, tag="w", name=f"w3_{k}")
                nc.sync.dma_start(w_k[:], W3[k * P : (k + 1) * P, :])
                w3t.append(w_k)
            for k in range(KT):
                nc.sync.dma_start(vtt[k][:], VT[k * P : (k + 1) * P, :])

            # --- layer 1: k-outer over jt-groups of 4 (8 open PSUM banks) --
            # The PE starts on k-tile 0 as soon as it lands and absorbs each
            # later k-tile as it arrives; a jt-major loop would stall until
            # ALL of x+W1 were resident.
            h1 = [None] * JT
            for grp in range(JT // 4):
                jts = list(range(4 * grp, 4 * grp + 4))
                ps = {}
                for jt in jts:
                    ps[jt] = [
                        pp.tile([P, NB], F32, tag="ps", name=f"l1ps{jt}_{c}")
                        for c in range(NBC)
                    ]
                for k in range(KT1):
                    for jt in jts:
                        lhsT = w1t[k][:, jt * P : (jt + 1) * P]
                        for c in range(NBC):
                            nc.tensor.matmul(
                                ps[jt][c][:],
                                lhsT,
                                xt[k][:, c * NB : (c + 1) * NB],
                                start=(k == 0),
                                stop=(k == KT1 - 1),
                            )
                for jt in jts:
                    ht = hpool.tile([P, BC], BF16, tag="h", name=f"l1h{jt}")
                    for c in range(NBC):
                        nc.scalar.activation(
                            ht[:, c * NB : (c + 1) * NB],
                            ps[jt][c][:],
                            AF.Tanh,
                            bias=b1t[:, jt : jt + 1],
                        )
                    h1[jt] = ht


            # --- layers 2/3: jt-outer with rotated k-accumulation ----------
            # L2's base k-order follows W2 DMA arrival (gpsimd's k15..k13
            # land first, then sync's k0..k12) so jt=0 never waits.
            L2_BASE = [15, 14, 13] + list(range(13))

            def layer(h_prev, w_tiles, bias_t, ktiles, name, base=None):
                if base is None:
                    base = list(range(ktiles))
                h_out = []
                for jt in range(JT):
                    ps = []
                    for c in range(NBC):
                        ps_c = pp.tile([P, NB], F32, tag="ps", name=f"{name}ps{jt}_{c}")
                        ps.append(ps_c)
                    # Rotate the accumulation order by jt so each weight
                    # tile's final read retires early for some jt, releasing
                    # its pool slot for the next layer's prefetch DMA.
                    kts = [base[(i + jt) % ktiles] for i in range(ktiles)]
                    for i, kt in enumerate(kts):
                        lhsT = w_tiles[kt][:, jt * P : (jt + 1) * P]
                        for c in range(NBC):
                            nc.tensor.matmul(
                                ps[c][:],
                                lhsT,
                                h_prev[kt][:, c * NB : (c + 1) * NB],
                                start=(i == 0),
                                stop=(i == ktiles - 1),
                            )
                    ht = hpool.tile([P, BC], BF16, tag="h", name=f"{name}h{jt}")
                    for c in range(NBC):
                        nc.scalar.activation(
                            ht[:, c * NB : (c + 1) * NB],
                            ps[c][:],
                            AF.Tanh,
                            bias=bias_t[:, jt : jt + 1],
                        )
                    h_out.append(ht)
                return h_out

            h2 = layer(h1, w2t, b2t, KT, "l2", base=L2_BASE)
            h3 = layer(h2, w3t, b3t, KT, "l3")

            # --- h3 squared (stationary operand for the diag matmuls) -----
            h3sq = []
            for k in range(KT):
                sq_k = hpool.tile([P, BC], BF16, tag="h", name=f"h3sq{k}")
                nc.vector.tensor_mul(sq_k[:], h3[k][:], h3[k][:])
                h3sq.append(sq_k)

            # --- FM stage: per 128-row batch tile -------------------------
            def fm_phase_a(bt):
                """vx = h V^T (1024 cols) and lin = h fm_w^T (64 cols)."""
                vx0 = pp.tile([P, NB], F32, tag="ps", name=f"vx0_{bt}")
                vx1 = pp.tile([P, NB], F32, tag="ps", name=f"vx1_{bt}")
                lw = pp.tile([P, NB], F32, tag="ps", name=f"lw_{bt}")
                bsl = slice(bt * P, (bt + 1) * P)
                for kt in range(KT):
                    lhsT = h3[kt][:, bsl]
                    nc.tensor.matmul(
                        vx0[:], lhsT, vtt[kt][:, 0:NB],
                        start=(kt == 0), stop=(kt == KT - 1),
                    )
                    nc.tensor.matmul(
                        vx1[:], lhsT, vtt[kt][:, NB:HR],
                        start=(kt == 0), stop=(kt == KT - 1),
                    )
                    nc.tensor.matmul(
                        lw[:, 0:HEADS], lhsT,
                        fwt[:, kt * HEADS : (kt + 1) * HEADS],
                        start=(kt == 0), stop=(kt == KT - 1),
                    )
                return vx0, vx1, lw

            def fm_phase_b(bt):
                """diag = (h*h) . (0.5 * sum_r V^2), already scaled by 0.5."""
                dg = pp.tile([P, NB], F32, tag="ps", name=f"dg_{bt}")
                bsl = slice(bt * P, (bt + 1) * P)
                for kt in range(KT):
                    nc.tensor.matmul(
                        dg[:, 0:HEADS],
                        h3sq[kt][:, bsl],
                        sqt[:, kt * HEADS : (kt + 1) * HEADS],
                        start=(kt == 0), stop=False,
                    )
                nc.tensor.matmul(
                    dg[:, 0:HEADS], onest[:], w0c[:], start=False, stop=True,
                )
                return dg

            def fm_square_reduce(bt, vx0, vx1):
                """Emitted right after phase A: overlaps later bt's matmuls.
                Each 512-wide half squares then reduces independently so the
                two chains pipeline across ACT and DVE."""
                vx2 = epool.tile([P, HR], F32, tag="e", name=f"vx2_{bt}")
                sumv = spool.tile([P, HEADS], F32, tag="s", name=f"sumv_{bt}")
                for c, vxh in ((0, vx0), (1, vx1)):
                    nc.scalar.activation(vx2[:, c * NB : (c + 1) * NB], vxh[:], AF.Square)
                    nc.vector.reduce_sum(
                        sumv[:, c * (HEADS // 2) : (c + 1) * (HEADS // 2)],
                        vx2[:, c * NB : (c + 1) * NB].rearrange(
                            "p (h r) -> p h r", r=RANK
                        ),
                        axis=mybir.AxisListType.X,
                    )
                return sumv

            def fm_combine(bt, sumv, lw, dg):
                # q = 0.5*sumv - diag_half
                q = spool.tile([P, HEADS], F32, tag="s", name=f"q_{bt}")
                nc.vector.scalar_tensor_tensor(
                    q[:], sumv[:], 0.5, dg[:, 0:HEADS],
                    op0=ALU.mult, op1=ALU.subtract,
                )
                ot = opool.tile([P, HEADS], F32, tag="o", name=f"ot_{bt}")
                nc.vector.tensor_add(ot[:], q[:], lw[:, 0:HEADS])
                nc.sync.dma_start(OUT[bt * P : (bt + 1) * P, :], ot[:])

            # Stagger: A(0), A(1), B(0), C(0), A(2), B(1), C(1), ...
            pend = []  # (bt, sumv, lw)
            for bt in range(BT):
                vx0, vx1, lw = fm_phase_a(bt)
                sumv = fm_square_reduce(bt, vx0, vx1)
                pend.append((bt, sumv, lw))
                if len(pend) == 2:
                    obt, osumv, olw = pend.pop(0)
                    dg = fm_phase_b(obt)
                    fm_combine(obt, osumv, olw, dg)
            while pend:
                obt, osumv, olw = pend.pop(0)
                dg = fm_phase_b(obt)
                fm_combine(obt, osumv, olw, dg)

    nc.compile()
    return nc


def _get_nc():
    if "nc" not in _CACHE:
        _CACHE["nc"] = _build_module()
    return _CACHE["nc"]


def _prep_host(x, W1, b1, W2, b2, W3, b3, fm_w0, fm_w, fm_V):
    """Host-side layout prep: bf16 casts, transposes, per-head V reductions."""
    bf = ml_dtypes.bfloat16
    f32 = np.float32

    common = {
        # column-group-major: row g*512 + r = W1[r, g*512:(g+1)*512]
        "W1": np.ascontiguousarray(
            W1.reshape(IN, 4, HID // 4).transpose(1, 0, 2).reshape(4 * IN, HID // 4)
            .astype(bf)
        ),
        "W2": np.ascontiguousarray(W2.astype(bf)),
        "W3": np.ascontiguousarray(W3.astype(bf)),
        "B1": np.ascontiguousarray(b1.astype(f32).reshape(JT, P).T),
        "B2": np.ascontiguousarray(b2.astype(f32).reshape(JT, P).T),
        "B3": np.ascontiguousarray(b3.astype(f32).reshape(JT, P).T),
        # V^T: [2048, heads*rank], col hr = h*RANK + r
        "VT": np.ascontiguousarray(
            fm_V.reshape(HEADS * RANK, HID).T.astype(bf)
        ),
        # fm_w^T packed as [128, kt*64]: FW[p, kt*64+h] = fm_w[h, kt*128+p]
        "FW": np.ascontiguousarray(
            fm_w.T.reshape(KT, P, HEADS).transpose(1, 0, 2).reshape(P, KT * HEADS)
            .astype(bf)
        ),
        # 0.5 * sum_r V^2, same packing
        "SQ": np.ascontiguousarray(
            (0.5 * (fm_V.astype(np.float64) ** 2).sum(axis=1))
            .T.reshape(KT, P, HEADS).transpose(1, 0, 2).reshape(P, KT * HEADS)
            .astype(bf)
        ),
        "W0C": np.ascontiguousarray(
            np.tile((-fm_w0.astype(np.float64) / P)[None, :], (P, 1))
            .astype(ml_dtypes.bfloat16)
        ),
    }

    in_maps = []
    xb = x.astype(bf)
    for c in range(NCORES):
        m = dict(common)
        m["xT"] = np.ascontiguousarray(xb[c * BC : (c + 1) * BC, :].T)
        in_maps.append(m)
    return in_maps


def kernel(x, W1, b1, W2, b2, W3, b3, fm_w0, fm_w, fm_V):
    # Host prep is plain numpy; coerce eagerly in case inputs are jax arrays.
    x, W1, b1, W2, b2, W3, b3, fm_w0, fm_w, fm_V = (
        np.asarray(a) for a in (x, W1, b1, W2, b2, W3, b3, fm_w0, fm_w, fm_V)
    )
    nc = _get_nc()
    in_maps = _prep_host(x, W1, b1, W2, b2, W3, b3, fm_w0, fm_w, fm_V)
    import os
    trace = bool(int(os.environ.get("KERNEL_TRACE", "0")))
    last_err = None
    for _attempt in range(3):
        try:
            res = bass_utils.run_bass_kernel_spmd(
                nc, in_maps, core_ids=list(range(NCORES)), trace=trace,
            )
            outs = [np.asarray(res.results[c]["out"]) for c in range(NCORES)]
            break
        except Exception as e:  # transient device faults (NRT unrecoverable)
            last_err = e
    else:
        raise last_err
    _CACHE["last_results"] = res
    full = np.concatenate(outs, axis=0)          # [B, HEADS]
    return np.ascontiguousarray(full.T).astype(np.float32)  # [HEADS, B]


# revision 17
# speedup vs baseline: 1.2010x; 1.0026x over previous
"""Trainium2 Bass kernel for NNBlendFM: 3-layer tanh MLP embedder + 64-head
rank-16 factorization machine, data-parallel over batch across 8 NeuronCores.

Math (per batch row b, head h):
    h = tanh(tanh(tanh(x W1 + b1) W2 + b2) W3 + b3)          # [B, 2048]
    lin[b,h]  = h . fm_w[h]
    vx[b,h,r] = h . fm_V[h,r]
    diag[b,h] = (h*h) . (sum_r fm_V[h,r]^2)
    out[h,b]  = fm_w0[h] + lin + 0.5*(sum_r vx^2 - diag)

Device layout: activations kept as [feature_partition, batch_free] tiles so
every matmul contracts over the partition dim with natural-layout weights as
the stationary operand.  The FM stage flips to [batch_partition, col_free] by
using h^T k-tiles as the stationary operand.  All matmul inputs are bf16
(fp32 PSUM accumulation), everything else fp32.

Head schedule: the kernel is HBM-latency-bound for its first ~16us (x+W1 are
3MB and DMA aggregate is ~410 GB/s, with ~8.4us of fixed prologue+queue
spin-up before the first packet lands).  To hide it, x and W1 are split into
24 x 128KB row-chunks issued k-ascending round-robin over four engine DMA
queues, and layer 1 runs k-outer over jt-groups of 4 (8 open PSUM banks) so
the PE consumes each k-tile as it lands instead of waiting for all of them.
"""

import numpy as np
import ml_dtypes

import concourse.tile as tile
from concourse import bacc, mybir
from concourse import bass_utils

BF16 = mybir.dt.bfloat16
F32 = mybir.dt.float32
AF = mybir.ActivationFunctionType
ALU = mybir.AluOpType

P = 128
IN, HID, HEADS, RANK = 512, 2048, 64, 16
B = 8192
NCORES = 8
BC = B // NCORES            # 1024 batch rows per core
KT1 = IN // P               # 4  k-tiles, layer 1
KT = HID // P               # 16 k-tiles, layers 2/3 + FM
JT = HID // P               # 16 output-feature tiles per layer
NB = 512                    # matmul moving free-dim (one PSUM bank)
NBC = BC // NB              # 2 batch column chunks
BT = BC // P                # 8 batch tiles in FM stage
HR = HEADS * RANK           # 1024 vx columns
WARMUP_MM = 8               # PE warm-up matmuls (HAM ramp coverage)

_CACHE = {}


def _build_module():
    nc = bacc.Bacc(
        "TRN2", target_bir_lowering=False, debug=False, num_devices=NCORES
    )
    dt = nc.dram_tensor
    xT = dt("xT", [IN, BC], BF16, kind="ExternalInput").ap()
    # W1 in column-group-major DRAM layout: row g*IN + r holds
    # W1[r, g*512 : (g+1)*512] so the (k, g) chunk the head streams is a
    # contiguous 128KB block (4KB coalesced packets).
    W1 = dt("W1", [4 * IN, HID // 4], BF16, kind="ExternalInput").ap()
    W2 = dt("W2", [HID, HID], BF16, kind="ExternalInput").ap()
    W3 = dt("W3", [HID, HID], BF16, kind="ExternalInput").ap()
    B1 = dt("B1", [P, JT], F32, kind="ExternalInput").ap()
    B2 = dt("B2", [P, JT], F32, kind="ExternalInput").ap()
    B3 = dt("B3", [P, JT], F32, kind="ExternalInput").ap()
    VT = dt("VT", [HID, HR], BF16, kind="ExternalInput").ap()
    FW = dt("FW", [P, KT * HEADS], BF16, kind="ExternalInput").ap()
    SQ = dt("SQ", [P, KT * HEADS], BF16, kind="ExternalInput").ap()
    W0C = dt("W0C", [P, HEADS], BF16, kind="ExternalInput").ap()
    OUT = dt("out", [BC, HEADS], F32, kind="ExternalOutput").ap()

    with tile.TileContext(nc) as tc:
        with (
            tc.tile_pool(name="wpool", bufs=24) as wpool,
            tc.tile_pool(name="hpool", bufs=32) as hpool,
            tc.tile_pool(name="vtpool", bufs=16) as vtpool,
            tc.tile_pool(name="cpool", bufs=1) as cpool,
            tc.tile_pool(name="pp", bufs=8, space="PSUM") as pp,
            tc.tile_pool(name="epool", bufs=2) as epool,
            tc.tile_pool(name="spool", bufs=8) as spool,
            tc.tile_pool(name="opool", bufs=4) as opool,
        ):
            # PE warm-up: dummy matmuls on a zeroed borrowed tile keep the PE
            # busy through the DMA head so HAM un-throttles (1.2 -> 2.4 GHz)
            # before the first real matmul.  vt0 is borrowed — its real DMA
            # fill happens mid-kernel, long after the warm-up reads.
            vtt = []
            for k in range(KT):
                vt_k = vtpool.tile([P, HR], BF16, tag="vt", name=f"vt{k}")
                vtt.append(vt_k)
            wsrc = vtt[0][:, 0:NB]
            nc.gpsimd.memset(wsrc, 0.0)
            wu = pp.tile([P, NB], F32, tag="ps", name="warm")
            for _ in range(WARMUP_MM):
                nc.tensor.matmul(
                    wu[:], wsrc[:, 0:P], wsrc[:], start=True, stop=True
                )

            # --- critical-path head DMA: x + W1 as 24 x 128KB chunks -------
            # Only sync/scalar/gpsimd can issue DMAs; sync+scalar are the
            # fast HWDGE queues, gpsimd's software DGE starts ~2us later and
            # sustains less.  Layer 1's group-0 critical set is only x (1MB)
            # + W1's g=0 column blocks (512KB) thanks to the g-major W1
            # layout; its 6 chunks ride sync+scalar in consumption order so
            # k-tile i of group 0 is resident ~1.7us after k-tile i-1.
            # b1 goes FIRST on gpsimd (8KB) so the first tanh never waits.
            b1t = cpool.tile([P, JT], F32, tag="b1")
            nc.gpsimd.dma_start(b1t[:], B1)
            onest = cpool.tile([P, P], BF16, tag="ones")
            nc.gpsimd.memset(onest[:], 1.0)
            xt = []
            w1t = []
            for k in range(KT1):
                x_k = hpool.tile([P, BC], BF16, tag="h", name=f"xt{k}")
                xt.append(x_k)
                w_k = wpool.tile([P, HID], BF16, tag="w", name=f"w1_{k}")
                w1t.append(w_k)

            def w1_chunk(eng, k, g):
                eng.dma_start(
                    w1t[k][:, g * 512 : (g + 1) * 512],
                    W1[g * IN + k * P : g * IN + (k + 1) * P, :],
                )

            # g=0 critical chunks, alternating queues per k.
            nc.sync.dma_start(xt[0][:], xT[0:P, :])
            w1_chunk(nc.scalar, 0, 0)
            w1_chunk(nc.sync, 1, 0)
            nc.scalar.dma_start(xt[1][:], xT[P : 2 * P, :])
            nc.sync.dma_start(xt[2][:], xT[2 * P : 3 * P, :])
            w1_chunk(nc.scalar, 2, 0)
            w1_chunk(nc.sync, 3, 0)
            nc.scalar.dma_start(xt[3][:], xT[3 * P : 4 * P, :])
            # W1 g=1 split sync/scalar (lands ~15.5us, consumed ~18us);
            # g=2/3 ride gpsimd's early window (lands ~14-18us).
            w1_chunk(nc.scalar, 0, 1)
            w1_chunk(nc.scalar, 1, 1)
            w1_chunk(nc.sync, 2, 1)
            w1_chunk(nc.sync, 3, 1)
            # b2 + W2 k12 on scalar (done issuing ~14us, before tanh g0 at
            # ~16); scalar then stays ACT-only for tanh.
            b2t = cpool.tile([P, JT], F32, tag="b2")
            nc.scalar.dma_start(b2t[:], B2)
            w2t = [None] * KT
            w2t[12] = wpool.tile([P, HID], BF16, tag="w", name="w2_12")
            nc.scalar.dma_start(w2t[12][:], W2[12 * P : 13 * P, :])

            # gpsimd: interleave its W2 share with the g2/g3 W1 blocks so
            # only ~0.5MB moves during the critical head window and each
            # piece still beats its deadline (g2 ~25us, g3 ~32us, W2 ~40us).
            def w2_load(eng, k):
                w_k = wpool.tile([P, HID], BF16, tag="w", name=f"w2_{k}")
                eng.dma_start(w_k[:], W2[k * P : (k + 1) * P, :])
                w2t[k] = w_k

            w2_load(nc.gpsimd, 15)
            for k in range(KT1):
                w1_chunk(nc.gpsimd, k, 2)
            w2_load(nc.gpsimd, 14)
            for k in range(KT1):
                w1_chunk(nc.gpsimd, k, 3)
            w2_load(nc.gpsimd, 13)
            # Remaining W2 on sync right behind its 1.5MB head (~17-40us).
            # L2 consumes k in arrival order (see L2_BASE below).
            for k in range(12):
                w2_load(nc.sync, k)
            # Small FM constants finish gpsimd's share (needed ~257us).
            b3t = cpool.tile([P, JT], F32, tag="b3")
            nc.gpsimd.dma_start(b3t[:], B3)
            # -w0/128 replicated; contracted against a ones column block so
            # the diag PSUM group finishes as (0.5*diag - w0).
            w0c = cpool.tile([P, HEADS], BF16, tag="w0c")
            nc.gpsimd.dma_start(w0c[:], W0C)
            fwt = cpool.tile([P, KT * HEADS], BF16, tag="fw")
            nc.gpsimd.dma_start(fwt[:], FW)
            sqt = cpool.tile([P, KT * HEADS], BF16, tag="sq")
            nc.gpsimd.dma_start(sqt[:], SQ)
            # W3 then VT on sync: W3 lands ~40-60us (needed from ~148us),
            # VT ~60-80us (needed ~257us) — sync is the lone bulk queue by
            # then and sustains ~400 GB/s.
            w3t = []
            for k in range(KT):
                w_k = wpool.tile([P, HID], BF16, tag="w", name=f"w3_{k}")
                nc.sync.dma_start(w_k[:], W3[k * P : (k + 1) * P, :])
                w3t.append(w_k)
            for k in range(KT):
                nc.sync.dma_start(vtt[k][:], VT[k * P : (k + 1) * P, :])

            # --- layer 1: k-outer over jt-groups of 4 (8 open PSUM banks) --
            # The PE starts on k-tile 0 as soon as it lands and absorbs each
            # later k-tile as it arrives; a jt-major loop would stall until
            # ALL of x+W1 were resident.
            h1 = [None] * JT
            for grp in range(JT // 4):
                jts = list(range(4 * grp, 4 * grp + 4))
                ps = {}
                for jt in jts:
                    ps[jt] = [
                        pp.tile([P, NB], F32, tag="ps", name=f"l1ps{jt}_{c}")
                        for c in range(NBC)
                    ]
                for k in range(KT1):
                    for jt in jts:
                        lhsT = w1t[k][:, jt * P : (jt + 1) * P]
                        for c in range(NBC):
                            nc.tensor.matmul(
                                ps[jt][c][:],
                                lhsT,
                                xt[k][:, c * NB : (c + 1) * NB],
                                start=(k == 0),
                                stop=(k == KT1 - 1),
                            )
                for jt in jts:
                    ht = hpool.tile([P, BC], BF16, tag="h", name=f"l1h{jt}")
                    for c in range(NBC):
                        nc.scalar.activation(
                            ht[:, c * NB : (c + 1) * NB],
                            ps[jt][c][:],
                            AF.Tanh,
                            bias=b1t[:, jt : jt + 1],
                        )
                    h1[jt] = ht


            # --- layers 2/3: jt-outer with rotated k-accumulation ----------
            # L2's base k-order follows W2 DMA arrival (gpsimd's k15..k13
            # land first, then sync's k0..k12) so jt=0 never waits.
            L2_BASE = [15, 14, 13] + list(range(13))

            def layer(h_prev, w_tiles, bias_t, ktiles, name, base=None):
                if base is None:
                    base = list(range(ktiles))
                h_out = []
                for jt in range(JT):
                    ps = []
                    for c in range(NBC):
                        ps_c = pp.tile([P, NB], F32, tag="ps", name=f"{name}ps{jt}_{c}")
                        ps.append(ps_c)
                    # Rotate the accumulation order by jt so each weight
                    # tile's final read retires early for some jt, releasing
                    # its pool slot for the next layer's prefetch DMA.
                    kts = [base[(i + jt) % ktiles] for i in range(ktiles)]
                    for i, kt in enumerate(kts):
                        lhsT = w_tiles[kt][:, jt * P : (jt + 1) * P]
                        for c in range(NBC):
                            nc.tensor.matmul(
                                ps[c][:],
                                lhsT,
                                h_prev[kt][:, c * NB : (c + 1) * NB],
                                start=(i == 0),
                                stop=(i == ktiles - 1),
                            )
                    ht = hpool.tile([P, BC], BF16, tag="h", name=f"{name}h{jt}")
                    for c in range(NBC):
                        nc.scalar.activation(
                            ht[:, c * NB : (c + 1) * NB],
                            ps[c][:],
                            AF.Tanh,
                            bias=bias_t[:, jt : jt + 1],
                        )
                    h_out.append(ht)
                return h_out

            h2 = layer(h1, w2t, b2t, KT, "l2", base=L2_BASE)
            h3 = layer(h2, w3t, b3t, KT, "l3")

            # --- h3 squared (stationary operand for the diag matmuls) -----
            h3sq = []
            for k in range(KT):
                sq_k = hpool.tile([P, BC], BF16, tag="h", name=f"h3sq{k}")
                nc.vector.tensor_mul(sq_k[:], h3[k][:], h3[k][:])
                h3sq.append(sq_k)

            # --- FM stage: per 128-row batch tile -------------------------
            def fm_phase_a(bt):
                """vx = h V^T (1024 cols) and lin = h fm_w^T (64 cols)."""
                vx0 = pp.tile([P, NB], F32, tag="ps", name=f"vx0_{bt}")
                vx1 = pp.tile([P, NB], F32, tag="ps", name=f"vx1_{bt}")
                lw = pp.tile([P, NB], F32, tag="ps", name=f"lw_{bt}")
                bsl = slice(bt * P, (bt + 1) * P)
                for kt in range(KT):
                    lhsT = h3[kt][:, bsl]
                    nc.tensor.matmul(
                        vx0[:], lhsT, vtt[kt][:, 0:NB],
                        start=(kt == 0), stop=(kt == KT - 1),
                    )
                    nc.tensor.matmul(
                        vx1[:], lhsT, vtt[kt][:, NB:HR],
                        start=(kt == 0), stop=(kt == KT - 1),
                    )
                    nc.tensor.matmul(
                        lw[:, 0:HEADS], lhsT,
                        fwt[:, kt * HEADS : (kt + 1) * HEADS],
                        start=(kt == 0), stop=(kt == KT - 1),
                    )
                return vx0, vx1, lw

            def fm_phase_b(bt):
                """diag = (h*h) . (0.5 * sum_r V^2), already scaled by 0.5."""
                dg = pp.tile([P, NB], F32, tag="ps", name=f"dg_{bt}")
                bsl = slice(bt * P, (bt + 1) * P)
                for kt in range(KT):
                    nc.tensor.matmul(
                        dg[:, 0:HEADS],
                        h3sq[kt][:, bsl],
                        sqt[:, kt * HEADS : (kt + 1) * HEADS],
                        start=(kt == 0), stop=False,
                    )
                nc.tensor.matmul(
                    dg[:, 0:HEADS], onest[:], w0c[:], start=False, stop=True,
                )
                return dg

            def fm_square_reduce(bt, vx0, vx1):
                """Emitted right after phase A: overlaps later bt's matmuls.
                Each 512-wide half squares then reduces independently so the
                two chains pipeline across ACT and DVE."""
                vx2 = epool.tile([P, HR], F32, tag="e", name=f"vx2_{bt}")
                sumv = spool.tile([P, HEADS], F32, tag="s", name=f"sumv_{bt}")
                for c, vxh in ((0, vx0), (1, vx1)):
                    nc.scalar.activation(vx2[:, c * NB : (c + 1) * NB], vxh[:], AF.Square)
                    nc.vector.reduce_sum(
                        sumv[:, c * (HEADS // 2) : (c + 1) * (HEADS // 2)],
                        vx2[:, c * NB : (c + 1) * NB].rearrange(
                            "p (h r) -> p h r", r=RANK
                        ),
                        axis=mybir.AxisListType.X,
                    )
                return sumv

            def fm_combine(bt, sumv, lw, dg):
                # q = 0.5*sumv - diag_half
                q = spool.tile([P, HEADS], F32, tag="s", name=f"q_{bt}")
                nc.vector.scalar_tensor_tensor(
                    q[:], sumv[:], 0.5, dg[:, 0:HEADS],
                    op0=ALU.mult, op1=ALU.subtract,
                )
                ot = opool.tile([P, HEADS], F32, tag="o", name=f"ot_{bt}")
                nc.vector.tensor_add(ot[:], q[:], lw[:, 0:HEADS])
                nc.sync.dma_start(OUT[bt * P : (bt + 1) * P, :], ot[:])

            # Stagger: A(0), A(1), B(0), C(0), A(2), B(1), C(1), ...
            pend = []  # (bt, sumv, lw)
            for bt in range(BT):
                vx0, vx1, lw = fm_phase_a(bt)
                sumv = fm_square_reduce(bt, vx0, vx1)
                pend.append((bt, sumv, lw))
                if len(pend) == 2:
                    obt, osumv, olw = pend.pop(0)
                    dg = fm_phase_b(obt)
                    fm_combine(obt, osumv, olw, dg)
            while pend:
                obt, osumv, olw = pend.pop(0)
                dg = fm_phase_b(obt)
                fm_combine(obt, osumv, olw, dg)

    nc.compile()
    return nc


def _get_nc():
    if "nc" not in _CACHE:
        _CACHE["nc"] = _build_module()
    return _CACHE["nc"]


def _prep_host(x, W1, b1, W2, b2, W3, b3, fm_w0, fm_w, fm_V):
    """Host-side layout prep: bf16 casts, transposes, per-head V reductions."""
    bf = ml_dtypes.bfloat16
    f32 = np.float32

    common = {
        # column-group-major: row g*512 + r = W1[r, g*512:(g+1)*512]
        "W1": np.ascontiguousarray(
            W1.reshape(IN, 4, HID // 4).transpose(1, 0, 2).reshape(4 * IN, HID // 4)
            .astype(bf)
        ),
        "W2": np.ascontiguousarray(W2.astype(bf)),
        "W3": np.ascontiguousarray(W3.astype(bf)),
        "B1": np.ascontiguousarray(b1.astype(f32).reshape(JT, P).T),
        "B2": np.ascontiguousarray(b2.astype(f32).reshape(JT, P).T),
        "B3": np.ascontiguousarray(b3.astype(f32).reshape(JT, P).T),
        # V^T: [2048, heads*rank], col hr = h*RANK + r
        "VT": np.ascontiguousarray(
            fm_V.reshape(HEADS * RANK, HID).T.astype(bf)
        ),
        # fm_w^T packed as [128, kt*64]: FW[p, kt*64+h] = fm_w[h, kt*128+p]
        "FW": np.ascontiguousarray(
            fm_w.T.reshape(KT, P, HEADS).transpose(1, 0, 2).reshape(P, KT * HEADS)
            .astype(bf)
        ),
        # 0.5 * sum_r V^2, same packing
        "SQ": np.ascontiguousarray(
            (0.5 * (fm_V.astype(np.float64) ** 2).sum(axis=1))
            .T.reshape(KT, P, HEADS).transpose(1, 0, 2).reshape(P, KT * HEADS)
            .astype(bf)
        ),
        "W0C": np.ascontiguousarray(
            np.tile((-fm_w0.astype(np.float64) / P)[None, :], (P, 1))
            .astype(ml_dtypes.bfloat16)
        ),
    }

    in_maps = []
    xb = x.astype(bf)
    for c in range(NCORES):
        m = dict(common)
        m["xT"] = np.ascontiguousarray(xb[c * BC : (c + 1) * BC, :].T)
        in_maps.append(m)
    return in_maps


def kernel(x, W1, b1, W2, b2, W3, b3, fm_w0, fm_w, fm_V):
    # Host prep is plain numpy; coerce eagerly in case inputs are jax arrays.
    x, W1, b1, W2, b2, W3, b3, fm_w0, fm_w, fm_V = (
        np.asarray(a) for a in (x, W1, b1, W2, b2, W3, b3, fm_w0, fm_w, fm_V)
    )
    nc = _get_nc()
    in_maps = _prep_host(x, W1, b1, W2, b2, W3, b3, fm_w0, fm_w, fm_V)
    import os
    trace = bool(int(os.environ.get("KERNEL_TRACE", "0")))
    last_err = None
    for _attempt in range(3):
        try:
            res = bass_utils.run_bass_kernel_spmd(
                nc, in_maps, core_ids=list(range(NCORES)), trace=trace,
            )
            outs = [np.asarray(res.results[c]["out"]) for c in range(NCORES)]
            break
        except Exception as e:  # transient device faults (NRT unrecoverable)
            last_err = e
    else:
        raise last_err
    _CACHE["last_results"] = res
    full = np.concatenate(outs, axis=0)          # [B, HEADS]
    return np.ascontiguousarray(full.T).astype(np.float32)  # [HEADS, B]
